# revision 4
# baseline (speedup 1.0000x reference)
"""GCN AutoEncoder (6-layer, BN+ReLU) on 8 Trainium2 NeuronCores.

v2: replaces per-chunk indirect_dma_start gathers (~1us fixed SWDGE cost
x ~4800 chunks = the v1 bottleneck) with a few large dma_gather
(InstDMAGatherAnt) calls per layer.

Strategy (dst-sharded graph parallel):
  - nodes partitioned contiguously: core i owns rows [i*NPC, (i+1)*NPC)
  - per layer: local transform t = dis * (bn_fold(v) @ W) in bf16
    -> AllGather t (rows padded to 128 bf16 = 256B) -> h_all table in DRAM
    -> per group of dst tiles: 2 dma_gather calls (table rows < 32768 and
       >= 32768, since gather idxs are int16) pull all edge rows into SBUF
    -> per dst tile: one batched is_equal builds S[e,d] = (dst_rel[e]==d)
       for all chunks; PE matmul M.T @ S accumulated in PSUM
    -> self-loop via PE transpose of local t tile (no gather)
    -> epilogue: v = relu(dis*agg + b); BN folded into next W via stats
       AllReduce (mean/var -> per-feature affine applied on ACT)
Edge chunks are padded to 128-multiples per (dst-tile, lo/hi half) and
equalized across cores so the SPMD instruction stream is identical.
"""
import os
import sys

sys.path.insert(0, "/opt/trn_rl_repo")

import numpy as np
import ml_dtypes

import concourse.bass as bass
import concourse.mybir as mybir
import concourse.tile as tile
from concourse import bacc
from concourse.bass_utils import run_bass_kernel_spmd

F32 = mybir.dt.float32
BF16 = mybir.dt.bfloat16
I16 = mybir.dt.int16
AF = mybir.ActivationFunctionType
ALU = mybir.AluOpType

NCORES = 8
P = 128
ROWW = 128          # table row width (bf16 elems) -> 256B, dma_gather granule
LO_ROWS = 32768     # int16 idx limit: rows below go in the "lo" gather
GBLK = 96           # max gathered chunks per group (96*256B = 24KB/partition)


class Cfg:
    def __init__(self, n_nodes=50000, dims=None):
        self.n = n_nodes
        self.dims = dims or [(88, 70), (70, 60), (60, 50), (50, 60), (60, 70), (70, 88)]
        self.relu = [True, True, False, True, True, False]
        self.bn = [True, True, False, True, True, False]
        self.npc = self.n // NCORES
        assert self.npc * NCORES == self.n
        self.ntiles = (self.npc + P - 1) // P
        self.m_last = self.npc - (self.ntiles - 1) * P
        self.eps = 1e-5


def preprocess(cfg, x, edge_index):
    """Host-side: degrees/dis, per-(tile, lo/hi) chunked edge arrays laid out
    for dma_gather, plus node-major transform aux."""
    n, npc, ntiles = cfg.n, cfg.npc, cfg.ntiles
    src = np.asarray(edge_index[0], dtype=np.int64).astype(np.int32)
    dst = np.asarray(edge_index[1], dtype=np.int64).astype(np.int32)
    deg = np.bincount(dst, minlength=n).astype(np.float32) + 1.0  # + self loop
    dis = 1.0 / np.sqrt(deg)

    trows = ntiles * P  # table rows per rank block
    r = (src // npc) * trows + (src % npc)  # row in AllGather table
    core_of = dst // npc
    dloc = dst % npc
    tile_of = dloc // P
    drel_of = dloc - tile_of * P
    half_of = (r >= LO_ROWS).astype(np.int64)

    # per (core, tile, half) counts -> equalized chunk counts
    counts = np.zeros((NCORES, ntiles, 2), dtype=np.int64)
    np.add.at(counts, (core_of, tile_of, half_of), 1)
    assert counts.min() > 0, "empty (core,tile,half) bucket; padding logic assumes >0"
    cts = np.ceil(counts.max(axis=0) / P).astype(np.int64)  # [ntiles, 2]

    # groups of tiles, bounded by GBLK chunks
    groups = []
    cur, cur_blk = [], 0
    for t in range(ntiles):
        tb = int(cts[t, 0] + cts[t, 1])
        if cur and cur_blk + tb > GBLK:
            groups.append(cur)
            cur, cur_blk = [], 0
        cur.append(t)
        cur_blk += tb
    if cur:
        groups.append(cur)

    # chunk layout:
    #  - gather order (idx array): per group: [lo chunks of tiles in g][hi chunks]
    #  - drel order (S build): per tile contiguous: [lo chunks][hi chunks]
    # per tile: mg block index (group-relative) of chunk j, and drel col range
    tile_blk = {}       # t -> list of group-relative block indices (lo then hi)
    tile_drel_c0 = {}   # t -> first drel col
    group_meta = []     # per group: (tiles, nblk_lo, nblk_hi, idx_q0)
    idx_pos = {}        # (t, h) -> flat idx start position
    q = 0
    for g in groups:
        nblk_lo = int(sum(cts[t, 0] for t in g))
        nblk_hi = int(sum(cts[t, 1] for t in g))
        group_meta.append((g, nblk_lo, nblk_hi, q))
        off = 0
        for t in g:
            idx_pos[(t, 0)] = q + off * P
            tile_blk[t] = list(range(off, off + int(cts[t, 0])))
            off += int(cts[t, 0])
        for t in g:
            idx_pos[(t, 1)] = q + off * P
            tile_blk[t] += list(range(off, off + int(cts[t, 1])))
            off += int(cts[t, 1])
        q += (nblk_lo + nblk_hi) * P
    tot_idxs = q
    tot_chunks = tot_idxs // P

    c0 = 0
    for t in range(ntiles):
        tile_drel_c0[t] = c0
        c0 += int(cts[t, 0] + cts[t, 1])

    idx_flat = np.zeros((NCORES, tot_idxs), dtype=np.int16)
    drel = np.full((NCORES, P, tot_chunks), 200.0, dtype=np.float32)

    for i in range(NCORES):
        m = core_of == i
        ri, ti, hi_, di = r[m], tile_of[m], half_of[m], drel_of[m]
        order = np.lexsort((ri, hi_, ti))
        ri, ti, hi_, di = ri[order], ti[order], hi_[order], di[order]
        # running position within each (tile, half) bucket
        starts = np.zeros((ntiles, 2), dtype=np.int64)
        cnt = np.zeros((ntiles, 2), dtype=np.int64)
        np.add.at(cnt, (ti, hi_), 1)
        pos = np.zeros(len(ri), dtype=np.int64)
        # groups are sorted by (tile, half); compute run-relative index
        key = ti * 2 + hi_
        first = np.r_[True, key[1:] != key[:-1]]
        gstart = np.flatnonzero(first)
        pos = np.arange(len(key)) - np.repeat(
            gstart, np.diff(np.r_[gstart, len(key)]))
        # idx array position (gather order)
        base = np.array([idx_pos[(t, h)] for t, h in zip(ti[first], hi_[first])])
        flatp = np.repeat(base, np.diff(np.r_[gstart, len(key)])) + pos
        idx_flat[i, flatp] = (ri - hi_ * LO_ROWS).astype(np.int16)
        # drel position (per-tile order): col = drel_c0[t] + (h? ct_lo : 0) + pos//P
        dc0 = np.array([tile_drel_c0[t] + (int(cts[t, 0]) if h else 0)
                        for t, h in zip(ti[first], hi_[first])])
        dcol = np.repeat(dc0, np.diff(np.r_[gstart, len(key)])) + pos // P
        drel[i, pos % P, dcol] = di.astype(np.float32)

    # idx SBUF wrap: flat j -> partition j%16 (replicated x8), col j//16
    idx_cols = tot_idxs // 16
    idx_sb = np.zeros((NCORES, P, idx_cols), dtype=np.int16)
    for i in range(NCORES):
        w = idx_flat[i].reshape(idx_cols, 16).T  # [16, cols]
        idx_sb[i] = np.tile(w, (8, 1))

    # per-core node-major aux
    xs = np.asarray(x, dtype=np.float32)
    f_in0 = xs.shape[1]
    xT = np.zeros((NCORES, f_in0, trows), dtype=ml_dtypes.bfloat16)
    dis_col = np.zeros((NCORES, P, ntiles), dtype=np.float32)
    fmax = max(fo for _, fo in cfg.dims)
    dis_rep = np.zeros((NCORES, fmax, trows), dtype=np.float32)
    for i in range(NCORES):
        sl = slice(i * npc, (i + 1) * npc)
        xT[i, :, :npc] = xs[sl].T.astype(ml_dtypes.bfloat16)
        d = dis[sl]
        dis_col[i, : npc - (ntiles - 1) * P, ntiles - 1] = d[(ntiles - 1) * P:]
        for t in range(ntiles - 1):
            dis_col[i, :, t] = d[t * P:(t + 1) * P]
        dis_rep[i, :, :npc] = d[None, :]

    iota = np.tile(np.arange(P, dtype=np.float32), (P, 1))
    ident = np.eye(P, dtype=np.float32)

    return dict(
        idx_sb=idx_sb, drel=drel, cts=cts, groups=groups,
        group_meta=group_meta, tile_blk=tile_blk, tile_drel_c0=tile_drel_c0,
        tot_chunks=tot_chunks, tot_idxs=tot_idxs,
        xT=xT, dis_col=dis_col, dis_rep=dis_rep, iota=iota, ident=ident,
    )


def build_nc(cfg, pre):
    n, npc, ntiles, m_last = cfg.n, cfg.npc, cfg.ntiles, cfg.m_last
    dims = cfg.dims
    cts = pre["cts"]
    group_meta = pre["group_meta"]
    tile_blk = pre["tile_blk"]
    tile_drel_c0 = pre["tile_drel_c0"]
    tot_chunks = pre["tot_chunks"]
    tot_idxs = pre["tot_idxs"]
    trows = ntiles * P
    fmax = max(fo for _, fo in dims)
    f_in0 = dims[0][0]
    rg = [list(range(NCORES))]
    idx_cols = tot_idxs // 16

    nc = bacc.Bacc("TRN2", target_bir_lowering=False, debug=False,
                   num_devices=NCORES, num_swdge_queues=4)

    # ---- external IO
    xT_e = nc.dram_tensor("xT", [f_in0, trows], BF16, kind="ExternalInput")
    idx_e = nc.dram_tensor("idx", [P, idx_cols], I16, kind="ExternalInput")
    drel_e = nc.dram_tensor("drel", [P, tot_chunks], F32, kind="ExternalInput")
    iota_e = nc.dram_tensor("iota", [P, P], F32, kind="ExternalInput")
    ident_e = nc.dram_tensor("ident", [P, P], F32, kind="ExternalInput")
    dis_col_e = nc.dram_tensor("dis_col", [P, ntiles], F32, kind="ExternalInput")
    dis_rep_e = nc.dram_tensor("dis_rep", [fmax, trows], F32, kind="ExternalInput")
    b6_rep_e = nc.dram_tensor("b6_rep", [P, dims[5][1]], F32, kind="ExternalInput")
    w_e, b_e, g_e, be_e = [], [], [], []
    for l, (fi, fo) in enumerate(dims):
        w_e.append(nc.dram_tensor(f"W{l}", [fi, fo], BF16, kind="ExternalInput"))
        b_e.append(nc.dram_tensor(f"b{l}", [fo, 1], F32, kind="ExternalInput"))
        if cfg.bn[l]:
            g_e.append(nc.dram_tensor(f"g{l}", [fo, 1], F32, kind="ExternalInput"))
            be_e.append(nc.dram_tensor(f"be{l}", [fo, 1], F32, kind="ExternalInput"))
        else:
            g_e.append(None)
            be_e.append(None)
    out_e = nc.dram_tensor("out", [trows, dims[5][1]], F32, kind="ExternalOutput")

    with tile.TileContext(nc) as tc:
        with (
            tc.tile_pool(name="const", bufs=1) as cpool,
            tc.tile_pool(name="vt", bufs=2) as vtpool,
            tc.tile_pool(name="tsb", bufs=2) as tpool,
            tc.tile_pool(name="mg", bufs=2) as mpool,
            tc.tile_pool(name="ssb", bufs=4) as spool,
            tc.tile_pool(name="eps", bufs=4) as epool,
            tc.tile_pool(name="stat", bufs=2) as stpool,
            tc.tile_pool(name="psA", bufs=3, space="PSUM") as psA,
            tc.tile_pool(name="psB", bufs=2, space="PSUM") as psB,
            tc.tile_pool(name="psC", bufs=2, space="PSUM") as psC,
            tc.tile_pool(name="dram", bufs=1, space="DRAM") as dram,
        ):
            # ---- load constants to SBUF
            def load(pool, e, shape, dtype=F32):
                t = pool.tile(shape, dtype, name=f"c_{e.name}")
                nc.sync.dma_start(t[:], e[:])
                return t

            xT_sb = load(cpool, xT_e, [f_in0, trows], BF16)
            idx_sb = load(cpool, idx_e, [P, idx_cols], I16)
            drel_sb = load(cpool, drel_e, [P, tot_chunks])
            iota_sb = load(cpool, iota_e, [P, P])
            ident_sb = load(cpool, ident_e, [P, P])
            identb_sb = cpool.tile([P, P], BF16, name="identb")
            nc.vector.tensor_copy(identb_sb[:], ident_sb[:])
            dcol_sb = load(cpool, dis_col_e, [P, ntiles])
            drep_sb = load(cpool, dis_rep_e, [fmax, trows])
            b6r_sb = load(cpool, b6_rep_e, [P, dims[5][1]])
            w_sb = [load(cpool, w_e[l], [dims[l][0], dims[l][1]], BF16) for l in range(6)]
            b_sb = [load(cpool, b_e[l], [dims[l][1], 1]) for l in range(6)]
            g_sb = [load(cpool, g_e[l], [dims[l][1], 1]) if cfg.bn[l] else None for l in range(6)]
            be_sb = [load(cpool, be_e[l], [dims[l][1], 1]) if cfg.bn[l] else None for l in range(6)]

            # DRAM comm buffers (table rows padded to ROWW bf16 = 256B)
            ag_in = [dram.tile([trows, ROWW], BF16, tag=f"agin{l}", name=f"agin{l}") for l in range(6)]
            ag_out = [dram.tile([NCORES * trows, ROWW], BF16, tag=f"agout{l}", name=f"agout{l}", addr_space="Shared") for l in range(6)]
            ar_in = [dram.tile([dims[l][1], 2], F32, tag=f"arin{l}", name=f"arin{l}") if cfg.bn[l] else None for l in range(6)]
            ar_out = [dram.tile([dims[l][1], 2], F32, tag=f"arout{l}", name=f"arout{l}", addr_space="Shared") if cfg.bn[l] else None for l in range(6)]

            prev_vT = None       # [F_in, trows] bf16 post-activation (pre-bn)
            bn_cur = None        # (gs, cv) per-partition affine for pending bn

            for l in range(6):
                f_in, f_out = dims[l]
                tile_ms = [P] * (ntiles - 1) + [m_last]

                # ---------- transform: t = dis * (bn(v) @ W)  [node-major bf16]
                t_sb = tpool.tile([P, ntiles * ROWW], BF16, tag="tsb", name="tsb")
                for t in range(ntiles):
                    m = tile_ms[t]
                    lhsT = (xT_sb if l == 0 else prev_vT)[:f_in, t * P:t * P + m]
                    if bn_cur is not None:
                        gs_c, cv_c = bn_cur
                        vbn = epool.tile([fmax, P], BF16, tag="vbn", name="vbn")
                        nc.scalar.activation(vbn[:f_in, :m], lhsT, AF.Identity,
                                             bias=cv_c[:f_in, 0:1],
                                             scale=gs_c[:f_in, 0:1])
                        lhsT = vbn[:f_in, :m]
                    tps = psB.tile([P, f_out], F32, tag="tps", name="tps")
                    nc.tensor.matmul(tps[:m, :], lhsT=lhsT, rhs=w_sb[l][:f_in, :f_out],
                                     start=True, stop=True)
                    tsl = t_sb[:m, t * ROWW:t * ROWW + f_out]
                    nc.vector.tensor_scalar_mul(tsl, tps[:m, :], dcol_sb[:m, t:t + 1])
                nc.sync.dma_start(
                    ag_in[l][:].rearrange("(t p) f -> p t f", p=P),
                    t_sb[:].rearrange("p (t f) -> p t f", f=ROWW))

                # ---------- AllGather
                nc.gpsimd.collective_compute(
                    "AllGather", ALU.bypass,
                    ins=[ag_in[l][:].opt()],
                    outs=[ag_out[l][:].opt()],
                    replica_groups=rg,
                )

                # ---------- aggregation
                if cfg.bn[l]:
                    ssum = stpool.tile([f_out, ntiles], F32, tag="ssum", name="ssum")
                    ssq = stpool.tile([f_out, ntiles], F32, tag="ssq", name="ssq")
                if l < 5:
                    vT = vtpool.tile([fmax, trows], BF16, tag="vt", name="vt")

                for gi, (gtiles, nblk_lo, nblk_hi, idx_q0) in enumerate(group_meta):
                    nblk = nblk_lo + nblk_hi
                    mg = mpool.tile([P, nblk * ROWW], BF16, tag="mg", name="mg")
                    mg3 = mg[:].rearrange("p (b f) -> p b f", f=ROWW)
                    ic0 = idx_q0 // 16
                    if nblk_lo:
                        nc.gpsimd.dma_gather(
                            mg3[:, 0:nblk_lo, :], ag_out[l][0:LO_ROWS, :],
                            idx_sb[:, ic0:ic0 + nblk_lo * 8],
                            nblk_lo * P, nblk_lo * P, ROWW,
                            queue_num=(2 * gi) % 4, single_packet=False)
                    if nblk_hi:
                        nc.gpsimd.dma_gather(
                            mg3[:, nblk_lo:nblk, :],
                            ag_out[l][LO_ROWS:NCORES * trows, :],
                            idx_sb[:, ic0 + nblk_lo * 8:ic0 + nblk * 8],
                            nblk_hi * P, nblk_hi * P, ROWW,
                            queue_num=(2 * gi + 1) % 4, single_packet=False)

                    for t in gtiles:
                        m = tile_ms[t]
                        ct = int(cts[t, 0] + cts[t, 1])
                        tsl = t_sb[:m, t * ROWW:t * ROWW + f_out]
                        # batched S build: S[e, j*128+d] = (drel[e, c0+j] == d)
                        s = spool.tile([P, ct * P], BF16, tag="ssb", name="ssb")
                        dc0 = tile_drel_c0[t]
                        nc.vector.tensor_tensor(
                            out=s[:].rearrange("p (c d) -> p c d", d=P),
                            in0=drel_sb[:, dc0:dc0 + ct].unsqueeze(2)
                                .broadcast_to([P, ct, P]),
                            in1=iota_sb[:].unsqueeze(1).broadcast_to([P, ct, P]),
                            op=ALU.is_equal)
                        if l < 5:
                            selfT = psC.tile([f_out, P], BF16, tag="selfT", name="selfT")
                            nc.tensor.transpose(selfT[:f_out, :m], tsl, identb_sb[:m, :m])
                            agg = psA.tile([f_out, P], F32, tag="agg", name="agg")
                        else:
                            agg = psA.tile([P, f_out], F32, tag="agg", name="agg")
                        for j, blk in enumerate(tile_blk[t]):
                            lhsT = mg[:, blk * ROWW:blk * ROWW + f_out]
                            ssl = s[:, j * P:j * P + m]
                            if l < 5:
                                nc.tensor.matmul(agg[:f_out, :m], lhsT=lhsT,
                                                 rhs=ssl, start=(j == 0),
                                                 stop=(j == ct - 1))
                            else:
                                nc.tensor.matmul(agg[:m, :f_out], lhsT=ssl,
                                                 rhs=lhsT, start=(j == 0),
                                                 stop=(j == ct - 1))

                        if l < 5:
                            # epilogue: v = act(dis * (agg + selfT) + b)
                            selfT_sb = epool.tile([f_out, P], F32, tag="eself", name="eself")
                            nc.vector.tensor_copy(selfT_sb[:f_out, :m], selfT[:f_out, :m])
                            tmp = epool.tile([f_out, P], F32, tag="etmp", name="etmp")
                            nc.vector.tensor_tensor(out=tmp[:f_out, :m], in0=agg[:f_out, :m],
                                                    in1=selfT_sb[:f_out, :m], op=ALU.add)
                            tmp2 = epool.tile([f_out, P], F32, tag="etmp2", name="etmp2")
                            nc.vector.tensor_tensor(
                                out=tmp2[:f_out, :m], in0=tmp[:f_out, :m],
                                in1=drep_sb[:f_out, t * P:t * P + m], op=ALU.mult)
                            vsl = vT[:f_out, t * P:t * P + m]
                            nc.scalar.activation(
                                vsl, tmp2[:f_out, :m],
                                AF.Relu if cfg.relu[l] else AF.Identity,
                                bias=b_sb[l][:f_out, 0:1])
                            if cfg.bn[l]:
                                nc.vector.tensor_reduce(
                                    out=ssum[:f_out, t:t + 1], in_=vsl,
                                    axis=mybir.AxisListType.X, op=ALU.add)
                                sq = epool.tile([f_out, P], F32, tag="esq", name="esq")
                                nc.vector.tensor_tensor(out=sq[:f_out, :m], in0=vsl,
                                                        in1=vsl, op=ALU.mult)
                                nc.vector.tensor_reduce(
                                    out=ssq[:f_out, t:t + 1], in_=sq[:f_out, :m],
                                    axis=mybir.AxisListType.X, op=ALU.add)
                        else:
                            tmp = epool.tile([P, f_out], F32, tag="ftmp", name="ftmp")
                            nc.vector.tensor_tensor(out=tmp[:m, :], in0=agg[:m, :f_out],
                                                    in1=tsl, op=ALU.add)
                            tmp2 = epool.tile([P, f_out], F32, tag="ftmp2", name="ftmp2")
                            nc.vector.tensor_scalar_mul(tmp2[:m, :], tmp[:m, :],
                                                        dcol_sb[:m, t:t + 1])
                            osl = epool.tile([P, f_out], F32, tag="osl", name="osl")
                            nc.vector.tensor_tensor(out=osl[:m, :], in0=tmp2[:m, :],
                                                    in1=b6r_sb[:m, :f_out], op=ALU.add)
                            nc.sync.dma_start(out_e[t * P:t * P + m, :], osl[:m, :])

                # ---------- stats AllReduce + fold into next-layer affine
                if l < 5:
                    if cfg.bn[l]:
                        pack = stpool.tile([f_out, 2], F32, tag="pack", name="pack")
                        nc.vector.tensor_reduce(out=pack[:f_out, 0:1],
                                                in_=ssum[:f_out, :ntiles],
                                                axis=mybir.AxisListType.X, op=ALU.add)
                        nc.vector.tensor_reduce(out=pack[:f_out, 1:2],
                                                in_=ssq[:f_out, :ntiles],
                                                axis=mybir.AxisListType.X, op=ALU.add)
                        nc.sync.dma_start(ar_in[l][:], pack[:f_out, :])
                        nc.gpsimd.collective_compute(
                            "AllReduce", ALU.add,
                            ins=[ar_in[l][:].opt()],
                            outs=[ar_out[l][:].opt()],
                            replica_groups=rg,
                        )
                        st = stpool.tile([f_out, 2], F32, tag="st", name="st")
                        nc.sync.dma_start(st[:f_out, :], ar_out[l][:])
                        mu = stpool.tile([f_out, 1], F32, tag="mu", name="mu")
                        nc.vector.tensor_scalar_mul(mu[:f_out, :], st[:f_out, 0:1], 1.0 / cfg.n)
                        msq = stpool.tile([f_out, 1], F32, tag="msq", name="msq")
                        nc.vector.tensor_scalar_mul(msq[:f_out, :], st[:f_out, 1:2], 1.0 / cfg.n)
                        var = stpool.tile([f_out, 1], F32, tag="var", name="var")
                        nc.vector.tensor_tensor(out=var[:f_out, :], in0=mu[:f_out, :],
                                                in1=mu[:f_out, :], op=ALU.mult)
                        nc.vector.tensor_tensor(out=var[:f_out, :], in0=msq[:f_out, :],
                                                in1=var[:f_out, :], op=ALU.subtract)
                        nc.vector.tensor_scalar_add(var[:f_out, :], var[:f_out, :], cfg.eps)
                        rv = stpool.tile([f_out, 1], F32, tag="rv", name="rv")
                        nc.vector.reciprocal(rv[:f_out, :], var[:f_out, :])
                        rstd = stpool.tile([f_out, 1], F32, tag="rstd", name="rstd")
                        nc.scalar.activation(rstd[:f_out, :], rv[:f_out, :], AF.Sqrt)
                        gs = stpool.tile([f_out, 1], F32, tag="gs", name="gs")
                        nc.vector.tensor_tensor(out=gs[:f_out, :], in0=g_sb[l][:f_out, :],
                                                in1=rstd[:f_out, :], op=ALU.mult)
                        cv = stpool.tile([f_out, 1], F32, tag="cv", name="cv")
                        nc.vector.tensor_tensor(out=cv[:f_out, :], in0=gs[:f_out, :],
                                                in1=mu[:f_out, :], op=ALU.mult)
                        nc.vector.tensor_tensor(out=cv[:f_out, :], in0=be_sb[l][:f_out, :],
                                                in1=cv[:f_out, :], op=ALU.subtract)
                        bn_cur = (gs, cv)
                    else:
                        bn_cur = None
                    prev_vT = vT

    nc.compile()
    return nc


_CACHE = {}
LAST_RES = None


def _get_compiled(cfg, key, pre):
    if key not in _CACHE:
        _CACHE[key] = build_nc(cfg, pre)
    return _CACHE[key]


def _run(inputs, trace=False):
    cfg = Cfg(n_nodes=int(np.asarray(inputs["x"]).shape[0]))
    x = np.asarray(inputs["x"], dtype=np.float32)
    edge_index = np.asarray(inputs["edge_index"])
    pre = preprocess(cfg, x, edge_index)
    key = (cfg.n, edge_index.shape[1], hash(edge_index.tobytes()))
    nc = _get_compiled(cfg, key, pre)

    b6_rep = np.tile(np.asarray(inputs["b6"], dtype=np.float32)[None, :], (P, 1))
    bn_map = {0: "1", 1: "2", 3: "3", 4: "4"}
    in_maps = []
    for i in range(NCORES):
        m = {
            "xT": pre["xT"][i],
            "idx": pre["idx_sb"][i],
            "drel": pre["drel"][i],
            "iota": pre["iota"],
            "ident": pre["ident"],
            "dis_col": pre["dis_col"][i],
            "dis_rep": pre["dis_rep"][i],
            "b6_rep": b6_rep,
        }
        for l in range(6):
            m[f"W{l}"] = np.asarray(inputs[f"W{l+1}"], dtype=np.float32).astype(ml_dtypes.bfloat16)
            m[f"b{l}"] = np.asarray(inputs[f"b{l+1}"], dtype=np.float32)[:, None]
            if cfg.bn[l]:
                m[f"g{l}"] = np.asarray(inputs[f"g{bn_map[l]}"], dtype=np.float32)[:, None]
                m[f"be{l}"] = np.asarray(inputs[f"be{bn_map[l]}"], dtype=np.float32)[:, None]
        in_maps.append(m)

    res = run_bass_kernel_spmd(nc, in_maps, core_ids=list(range(NCORES)), trace=trace)
    global LAST_RES
    LAST_RES = res
    parts = [res.results[i]["out"][:cfg.npc] for i in range(NCORES)]
    out = np.concatenate(parts, axis=0)
    return out, res.exec_time_ns


def kernel(**inputs) -> np.ndarray:
    out, _ = _run(inputs, trace=False)
    return out


def kernel_traced(**inputs):
    # NTFF profile hook is registered at interpreter boot (antenv.axon_hooks)
    return _run(inputs, trace=True)
